# revision 43
# baseline (speedup 1.0000x reference)
"""Trainium2 Bass kernel for nn_AttnBlock: LayerNorm -> 16-head attention -> out-proj.

Full inputs in, full output out. Sharding: 8 cores = 2 batches x 4 head-groups
(4 heads per core). Each core computes LN + QKV (its 256 feature slice) +
attention for its 4 heads + a partial output projection; the host sums the 4
partials per batch and adds the output bias.

v5 schedule: one long pipeline, ScalarE (exp) is the pacer:
  - x^T chunks split across two DMA rings (sync/scalar), weights on a third
    (gpsimd) so input bandwidth is not serialized on one ring.
  - LN stats matmuls + x^2 (split ScalarE/DVE halves) pace with chunk arrival.
  - kbar[0]/qbar[0]-slab0 (with in-PSUM augmented chunk) and V' mains drain
    as early as possible; the V' augment+scale fixups are deferred into the
    attention phase as filler work (they only gate the AV matmuls, which lag).
  - attention per (head-pair, 512-q-slab, k-tile): both heads' scores in one
    [128,1024] f32 PSUM tile (2 banks), ONE exp ACTIVATE per k-tile with
    immediate 1/8 scale, AV lags 2 k-tiles.
  - all remaining projections (QK pair1, V' fixups, out-proj) issue as filler
    matmul groups inside the attention loop, sized ~2.5K cycles so they fit
    the per-k-tile PE slack without delaying the next score matmul.
"""

import os
from contextlib import ExitStack

import numpy as np

import concourse.bass as bass
import concourse.tile as tile
from concourse import bacc, mybir
from concourse.bass_utils import run_bass_kernel_spmd

F32 = mybir.dt.float32
BF16 = mybir.dt.bfloat16

B, L, D = 2, 2048, 1024
NH_TOT, HS = 16, 64
NCORES = 8
HPC = 4                  # heads per core
FPC = HPC * HS           # 256 features per core
P = 128
DCH = D // P             # 8 x^T chunks
KCH = DCH + 1            # +1 augmented chunk
QS = 512                 # q slab
NQS = L // QS            # 4
KT = L // P              # 16 k tiles
TT = L // P              # 16 token tiles
EPS = 1e-5
SCALE = float(HS) ** -0.5

LAST_RESULTS = None


def _build_nc():
    nc = bacc.Bacc("TRN2", target_bir_lowering=False, debug=False)

    xT = nc.dram_tensor("xT", [D, L], BF16, kind="ExternalInput").ap()
    wq = nc.dram_tensor("wq", [KCH * P, FPC], BF16, kind="ExternalInput").ap()
    wk = nc.dram_tensor("wk", [KCH * P, FPC], BF16, kind="ExternalInput").ap()
    wv = nc.dram_tensor("wv", [KCH * P, FPC], BF16, kind="ExternalInput").ap()
    wo = nc.dram_tensor("wo", [FPC, D], BF16, kind="ExternalInput").ap()
    out = nc.dram_tensor("out", [L, D], BF16, kind="ExternalOutput").ap()

    with tile.TileContext(nc) as tc, ExitStack() as ctx:
        persist = ctx.enter_context(tc.tile_pool(name="persist", bufs=1))

        # ---------------- persistent tiles ----------------
        eps_t = persist.tile([P, 1], F32, name="eps")
        nc.vector.memset(eps_t[:], EPS)
        ones_bf = persist.tile([P, 1], BF16, name="ones_bf")
        nc.vector.memset(ones_bf[:], 1.0)
        ones_row = persist.tile([1, P], BF16, name="ones_row")
        nc.vector.memset(ones_row[:], 1.0)
        dummy = persist.tile([P, 1], F32, name="dummy")

        xch = [persist.tile([P, L], BF16, name=f"x{c}") for c in range(DCH)]
        xch8 = persist.tile([P, L], BF16, name="x8")       # augmented rows
        r_bcast = persist.tile([P, L], F32, name="r_bcast")
        qbar = [persist.tile([P, L], BF16, name=f"qb{i}") for i in range(2)]
        kbar = [persist.tile([P, L], BF16, name=f"kb{i}") for i in range(2)]
        vprime = [persist.tile([P, HPC, HS + 2], BF16, name=f"vp{t}")
                  for t in range(TT)]
        onrm = [persist.tile([P, L], BF16, name=f"on{i}") for i in range(2)]
        r_cols = persist.tile([P, TT], F32, name="rcol")

        # x chunks alternate between two hardware DMA rings; weights go on a
        # third so x never waits behind them
        for c in range(DCH):
            eng = nc.sync if c % 2 == 0 else nc.scalar
            eng.dma_start(out=xch[c][:], in_=xT[P * c:P * (c + 1), :])
        wp = ctx.enter_context(tc.tile_pool(name="wp", bufs=1))
        wv_t = [wp.tile([P, FPC], BF16, name=f"wv{c}") for c in range(KCH)]
        wk_t = [wp.tile([P, FPC], BF16, name=f"wk{c}") for c in range(KCH)]
        wq_t = [wp.tile([P, FPC], BF16, name=f"wq{c}") for c in range(KCH)]
        wo_t = [wp.tile([P, D], BF16, name=f"wo{ch}") for ch in range(2)]
        # wk/wv (needed first, for kb0 and V' mains) ride the fast sync
        # hardware ring behind the x chunks; wq/wo (needed only mid-C as
        # filler inputs) stay on the slow gpsimd software DGE
        for c in range(KCH):
            nc.sync.dma_start(out=wk_t[c][:], in_=wk[P * c:P * (c + 1), :])
        for c in range(KCH):
            nc.sync.dma_start(out=wv_t[c][:], in_=wv[P * c:P * (c + 1), :])
        for c in range(KCH):
            nc.gpsimd.dma_start(out=wq_t[c][:], in_=wq[P * c:P * (c + 1), :])
        for ch in range(2):
            nc.gpsimd.dma_start(out=wo_t[ch][:], in_=wo[P * ch:P * (ch + 1), :])

        nc.vector.memset(xch8[:], 0.0)
        for t in range(TT):
            nc.vector.memset(vprime[t][:, :, HS:HS + 1], 1.0)
            nc.vector.memset(vprime[t][:, :, HS + 1:HS + 2], 0.0)

        rowstk = ExitStack()
        rowp = rowstk.enter_context(tc.tile_pool(name="rowp", bufs=1))
        musum_row = rowp.tile([1, L], F32, name="musum_row")
        sqsum_row = rowp.tile([1, L], F32, name="sqsum_row")
        r_row = rowp.tile([1, L], F32, name="r_row")
        r_row_bf = rowp.tile([1, L], BF16, name="r_row_bf")
        rows33 = rowp.tile([33, L], BF16, name="rows33")
        mu_row = rowp.tile([1, L], F32, name="mu_row")
        var_row = rowp.tile([1, L], F32, name="var_row")
        stdf_row = rowp.tile([1, L], F32, name="stdf_row")
        rscr_row = rowp.tile([1, L], F32, name="rscr_row")

        with ExitStack() as astk:
            statps = astk.enter_context(
                tc.tile_pool(name="statps", bufs=1, space="PSUM"))
            sqp = astk.enter_context(tc.tile_pool(name="sqp", bufs=2))

            # -------- phase A: LN stats, pipelined with x chunk arrival ----
            # x^2 split between ScalarE (Square) and DVE halves
            mps = statps.tile([1, L], F32, name="mps")
            sps_ = statps.tile([1, L], F32, name="sps_")
            for c in range(DCH):
                sq = sqp.tile([P, L], BF16, name="sqt")
                nc.scalar.activation(
                    out=sq[:, 0:1024], in_=xch[c][:, 0:1024],
                    func=mybir.ActivationFunctionType.Square, scale=1.0)
                nc.vector.tensor_mul(
                    sq[:, 1024:2048], xch[c][:, 1024:2048],
                    xch[c][:, 1024:2048])
                for s in range(L // 512):
                    sl = slice(512 * s, 512 * (s + 1))
                    nc.tensor.matmul(
                        mps[:, sl], ones_bf[:], xch[c][:, sl],
                        start=(c == 0), stop=(c == DCH - 1))
                    nc.tensor.matmul(
                        sps_[:, sl], ones_bf[:], sq[:, sl],
                        start=(c == 0), stop=(c == DCH - 1))
            nc.vector.tensor_copy(musum_row[:], mps[:])
            nc.vector.tensor_copy(sqsum_row[:], sps_[:])

        with ExitStack() as bstk:
            scrp = bstk.enter_context(
                tc.tile_pool(name="scrp", bufs=1, space="DRAM"))
            qkps = bstk.enter_context(
                tc.tile_pool(name="qkps", bufs=4, space="PSUM"))
            vpp = bstk.enter_context(
                tc.tile_pool(name="vpp", bufs=4, space="PSUM"))

            # ---- stats postprocess, row space ----
            nc.vector.tensor_scalar_mul(mu_row[:], musum_row[:], 1.0 / D)
            nc.vector.tensor_scalar_mul(rows33[0:1, :], musum_row[:], -1.0 / D)
            nc.vector.tensor_mul(var_row[:], mu_row[:], mu_row[:])
            nc.vector.tensor_scalar_mul(stdf_row[:], sqsum_row[:], 1.0 / D)
            nc.vector.tensor_sub(var_row[:], stdf_row[:], var_row[:])
            nc.scalar.activation(
                out=stdf_row[:], in_=var_row[:],
                func=mybir.ActivationFunctionType.Sqrt,
                bias=eps_t[0:1, :], scale=1.0)
            nc.scalar.activation(
                out=rows33[32:33, :], in_=var_row[:],
                func=mybir.ActivationFunctionType.Sqrt,
                bias=eps_t[0:1, :], scale=1.0)
            # preload the exp table set right after the last sqrt use
            nc.scalar.activation(
                out=dummy[:], in_=eps_t[:],
                func=mybir.ActivationFunctionType.Exp, scale=1.0)

            # ---- kbar[0] mains into held PSUM slots (aug + scaled drain
            # come once xch8 / r_bcast exist) ----
            kb0ps = [qkps.tile([P, 512], F32, name="kb0ps") for _ in range(4)]
            for s in range(4):
                sl = slice(512 * s, 512 * (s + 1))
                for c in range(DCH):
                    nc.tensor.matmul(
                        kb0ps[s][:], wk_t[c][:, 0:P], xch[c][:, sl],
                        start=(c == 0), stop=False)

            # ---- V' mains: 16 tiles stream through 4 packed banks, drained
            # RAW (aug+scale fixups happen later, as attention filler) ----
            vdrains = []
            for t in range(TT):
                if t % 2 == 0:
                    pv2 = vpp.tile([P, 2 * FPC], F32, name="pv2")
                pv = pv2[:, FPC * (t % 2):FPC * (t % 2 + 1)]
                for c in range(DCH):
                    nc.tensor.matmul(
                        pv, xch[c][:, P * t:P * (t + 1)], wv_t[c][:],
                        start=(c == 0 and t % 2 == 0),
                        stop=(c == DCH - 1 and t % 2 == 1))

                def vdrain(t=t, pv=pv):
                    nc.vector.tensor_copy(
                        vprime[t][:, :, 0:HS],
                        pv.rearrange("p (h f) -> p h f", h=HPC))
                vdrains.append(vdrain)
                if t == 3:
                    # slip the r chain in mid-stream so neither the V' ring
                    # nor the r consumers wait long
                    for vd in vdrains:
                        vd()
                    vdrains = []
                    nc.vector.reciprocal_approx_accurate(
                        out=r_row[:], in_=stdf_row[:], scratch=rscr_row[:])
                    nc.vector.tensor_copy(r_row_bf[:], r_row[:])
                    nc.gpsimd.dma_start(
                        out=xch8[0:2, :], in_=rows33[0:33:32, :])
                    scr_f = scrp.tile([1, L], F32, name="scr_f")
                    nc.gpsimd.dma_start(out=scr_f[0, :], in_=r_row[0:1, :])
                    nc.gpsimd.dma_start(
                        out=r_cols[:],
                        in_=scr_f[0, :].rearrange("(i p) -> p i", p=P))
            for vd in vdrains:
                vd()

            # ---- r broadcast (K=1 matmuls through freed kb0 ring slots
            # would deadlock: kb0 slots are still held, so use vpp ring) ----
            for s in range(4):
                sl = slice(512 * s, 512 * (s + 1))
                rb2 = vpp.tile([P, 2 * FPC], F32, name="pv2")
                nc.tensor.matmul(
                    rb2[:, 0:512], ones_row[:], r_row_bf[:, sl],
                    start=True, stop=True)
                nc.vector.tensor_copy(r_bcast[:, sl], rb2[:, 0:512])

            # ---- kbar[0] aug + scaled drains ----
            for s in range(4):
                sl = slice(512 * s, 512 * (s + 1))
                nc.tensor.matmul(
                    kb0ps[s][:], wk_t[DCH][:, 0:P], xch8[:, sl],
                    start=False, stop=True)
                nc.vector.tensor_mul(
                    kbar[0][:, sl], kb0ps[s][:], r_bcast[:, sl])

            # ---- qbar[0] slab 0 ----
            qps = qkps.tile([P, 512], F32, name="kb0ps")
            for c in range(KCH):
                rhs = xch[c] if c < DCH else xch8
                nc.tensor.matmul(
                    qps[:], wq_t[c][:, 0:P], rhs[:, 0:512],
                    start=(c == 0), stop=(c == KCH - 1))
            nc.vector.tensor_mul(
                qbar[0][:, 0:512], qps[:], r_bcast[:, 0:512])

        rowstk.close()

        # ----------------- filler work-list for attention phase -----------
        # small matmul groups issued inside the attention loop to fill the
        # PE slack under the exp-bound pacing; consumed strictly in order
        filler_units = []

        def vfix_unit(t0):
            # V' aug (via the zero-padded aug chunk) + (raw+aug)*r, 2 tiles
            def go(aux_pool):
                va2 = aux_pool.tile([P, 512], F32, name="aux")
                for t in (t0, t0 + 1):
                    nc.tensor.matmul(
                        va2[:, FPC * (t % 2):FPC * (t % 2 + 1)],
                        xch8[:, P * t:P * (t + 1)], wv_t[DCH][:],
                        start=(t % 2 == 0), stop=(t % 2 == 1))
                for t in (t0, t0 + 1):
                    va = va2[:, FPC * (t % 2):FPC * (t % 2 + 1)]
                    nc.vector.tensor_add(
                        vprime[t][:, :, 0:HS], vprime[t][:, :, 0:HS],
                        va.rearrange("p (h f) -> p h f", h=HPC))
                    nc.vector.tensor_scalar_mul(
                        vprime[t][:, :, 0:HS], vprime[t][:, :, 0:HS],
                        r_cols[:, t:t + 1])
            return go

        def qk_unit(wt, dst, m, s):
            # one 512-token slab of a q/k projection, split in three pumps
            cell = {}

            def mk(c0, c1, last):
                def go(aux_pool):
                    sl = slice(512 * s, 512 * (s + 1))
                    if c0 == 0:
                        cell["pq"] = aux_pool.tile([P, 512], F32, name="aux")
                    pq = cell["pq"]
                    for c in range(c0, c1):
                        rhs = xch[c] if c < DCH else xch8
                        nc.tensor.matmul(
                            pq[:], wt[c][:, P * m:P * (m + 1)], rhs[:, sl],
                            start=(c == 0), stop=(c == KCH - 1))
                    if last:
                        nc.vector.tensor_mul(
                            dst[m][:, sl], pq[:], r_bcast[:, sl])
                return go
            return mk(0, 3, False), mk(3, 6, False), mk(6, KCH, True)

        for t0 in range(0, TT, 2):
            filler_units.append((1300, vfix_unit(t0)))
        for m, wt, dst, s0 in ((0, wq_t, qbar, 1), (1, wk_t, kbar, 0),
                               (1, wq_t, qbar, 0)):
            for s in range(s0, 4):
                g1, g2, g3 = qk_unit(wt, dst, m, s)
                filler_units.append((1536, g1))
                filler_units.append((1536, g2))
                filler_units.append((1736, g3))

        def proj_unit(t, s2, use_scalar, late):
            def go(aux_pool):
                po = aux_pool.tile([P, 512], F32, name="aux")
                for ch in range(2):
                    nc.tensor.matmul(
                        po[:], onrm[ch][:, P * t:P * (t + 1)],
                        wo_t[ch][:, 512 * s2:512 * (s2 + 1)],
                        start=(ch == 0), stop=(ch == 1))
                ot = ostg_pool.tile([P, 512], BF16, name="ot")
                if use_scalar:
                    nc.scalar.copy(ot[:], po[:])
                else:
                    nc.vector.tensor_copy(ot[:], po[:])
                # late slabs ride the (idle) sync hardware ring so the final
                # dbc broadcasts aren't stuck behind gpsimd software copies
                eng = nc.sync if late else nc.gpsimd
                eng.dma_start(
                    out=out[P * t:P * (t + 1), 512 * s2:512 * (s2 + 1)],
                    in_=ot[:])
            return go

        with ExitStack() as cstk:
            spool = cstk.enter_context(
                tc.tile_pool(name="spool", bufs=2, space="PSUM"))
            opjp = cstk.enter_context(
                tc.tile_pool(name="opjp", bufs=1, space="PSUM"))
            auxp = cstk.enter_context(
                tc.tile_pool(name="auxp", bufs=2, space="PSUM"))
            epool = cstk.enter_context(tc.tile_pool(name="epool", bufs=6))
            ostg_pool = cstk.enter_context(tc.tile_pool(name="ostg", bufs=3))
            nrmp = cstk.enter_context(tc.tile_pool(name="nrmp", bufs=2))

            fill_i = 0
            fill_credit = 0

            def pump(budget_cycles):
                nonlocal fill_i, fill_credit
                fill_credit = min(fill_credit + budget_cycles, 20000)
                while fill_i < len(filler_units):
                    cost, go = filler_units[fill_i]
                    if cost > fill_credit:
                        break
                    go(auxp)
                    fill_i += 1
                    fill_credit -= cost

            # ---------------- phase C: attention ----------------
            # per k-tile: both heads' scores into one [128,1024] psum tile
            # (2 banks), ONE exp ACTIVATE straight from PSUM into bf16 SBUF,
            # AV lags 2 k-tiles so ScalarE paces and PE never blocks.
            AVLAG = 2
            for pair in range(2):
                qb, kb = qbar[pair], kbar[pair]
                for qs in range(NQS):
                    qsl = slice(QS * qs, QS * (qs + 1))
                    ops = [opjp.tile([HS + 2, QS], F32, name=f"op{ho}")
                           for ho in range(2)]
                    e_of = {}

                    def issue_av(kt):
                        for ho in range(2):
                            h = 2 * pair + ho
                            nc.tensor.matmul(
                                ops[ho][:],
                                vprime[kt][:, h, :],
                                e_of[kt][:, 512 * ho:512 * (ho + 1)],
                                start=(kt == 0),
                                stop=(kt == KT - 1))
                        del e_of[kt]

                    for kt in range(KT):
                        ksl = slice(P * kt, P * (kt + 1))
                        sp = spool.tile([P, 2 * QS], F32, name="sp")
                        for ho in range(2):
                            hb = HS * ho
                            nc.tensor.matmul(
                                sp[:, 512 * ho:512 * (ho + 1)],
                                kb[hb:hb + HS, ksl], qb[hb:hb + HS, qsl],
                                start=True, stop=True)
                        pump(1000 if pair == 0 else 1100)
                        if kt >= AVLAG:
                            issue_av(kt - AVLAG)
                        e = epool.tile([P, 2 * QS], BF16, name="e")
                        nc.scalar.activation(
                            out=e[:], in_=sp[:],
                            func=mybir.ActivationFunctionType.Exp,
                            scale=SCALE)
                        e_of[kt] = e
                    for kt in range(KT - AVLAG, KT):
                        issue_av(kt)

                    # softmax denominators -> normalize into onrm
                    for ho in range(2):
                        den0 = nrmp.tile([1, QS], F32, name="den0")
                        nc.vector.tensor_copy(den0[:], ops[ho][HS:HS + 1, :])
                        oraw = nrmp.tile([HS, QS], F32, name="oraw")
                        nc.vector.tensor_copy(oraw[:], ops[ho][0:HS, :])
                        dscr = nrmp.tile([1, QS], F32, name="dscr")
                        dinv = nrmp.tile([1, QS], F32, name="dinv")
                        nc.vector.reciprocal_approx_accurate(
                            out=dinv[:], in_=den0[:], scratch=dscr[:])
                        dbc = nrmp.tile([HS, QS], F32, name="dbc")
                        nc.gpsimd.partition_broadcast(dbc[:], dinv[:])
                        nc.vector.tensor_mul(
                            onrm[pair][HS * ho:HS * ho + HS, qsl],
                            oraw[:], dbc[:])

                    # out-projection for finished q-slabs becomes filler
                    if pair == 1:
                        for t in range(4 * qs, 4 * (qs + 1)):
                            for s2 in range(2):
                                filler_units.append(
                                    (1100, proj_unit(t, s2, qs == 3, qs >= 2)))
                        pump(1600)

            # drain any remaining filler (tail of the out-projection)
            while fill_i < len(filler_units):
                filler_units[fill_i][1](auxp)
                fill_i += 1

    nc.compile()
    return nc


_NC = None


def _host_weights(W, bias, ln_g, ln_b, rows):
    Wt = W * ln_g[None, :]
    c = W @ ln_b + bias
    s = Wt.sum(axis=1)
    What = np.zeros((KCH * P, FPC), np.float32)
    What[0:D, :] = Wt[rows].T
    What[D, :] = s[rows]
    What[D + 1, :] = c[rows]
    return What


def kernel(x, ln_g, ln_b, Wq, bq, Wk, bk, Wv, bv, Wo, bo):
    global _NC, LAST_RESULTS
    x = np.ascontiguousarray(np.asarray(x, np.float32))
    ln_g = np.asarray(ln_g, np.float32)
    ln_b = np.asarray(ln_b, np.float32)
    Wq, bq = np.asarray(Wq, np.float32), np.asarray(bq, np.float32)
    Wk, bk = np.asarray(Wk, np.float32), np.asarray(bk, np.float32)
    Wv, bv = np.asarray(Wv, np.float32), np.asarray(bv, np.float32)
    Wo, bo = np.asarray(Wo, np.float32), np.asarray(bo, np.float32)

    if _NC is None:
        _NC = _build_nc()

    import ml_dtypes
    bf = ml_dtypes.bfloat16
    in_maps = []
    for core in range(NCORES):
        b, g = core // HPC, core % HPC
        rows = slice(FPC * g, FPC * (g + 1))
        in_maps.append({
            "xT": np.ascontiguousarray(x[b].T).astype(bf),
            "wq": _host_weights(Wq, bq, ln_g, ln_b, rows).astype(bf),
            "wk": _host_weights(Wk, bk, ln_g, ln_b, rows).astype(bf),
            "wv": _host_weights(Wv, bv, ln_g, ln_b, rows).astype(bf),
            "wo": np.ascontiguousarray(Wo[:, rows].T).astype(bf),
        })

    res = run_bass_kernel_spmd(
        _NC, in_maps, core_ids=list(range(NCORES)),
        trace=bool(int(os.environ.get("KERNEL_TRACE", "0"))),
    )
    LAST_RESULTS = res

    out = np.zeros((B, L, D), np.float32)
    for b in range(B):
        acc = res.results[HPC * b]["out"].astype(np.float32).copy()
        for g in range(1, HPC):
            acc += res.results[HPC * b + g]["out"]
        out[b] = acc + bo[None, :]
    return out
